# revision 8
# baseline (speedup 1.0000x reference)
"""Multi-head causal self-attention (B=4, S=2048, D=1024, H=16) on 8 NeuronCores.

Sharding: core c handles batch b=c//2 and heads [8*(c%2), 8*(c%2)+8) (tensor
parallel over heads x data parallel over batch). Each core computes its 8
heads' Q/K/V projections, causal attention, and a partial O-projection
(contracting only its 512 ctx dims). Host sums the two partial outputs per
batch.

Per-core math / precision plan:
  Q/K/V projections in float32r (full PE rate, ~exact).
  Q/K evacuated to fp8e4m3 (x16); scores via fp8 DoubleRow matmuls (0.5
  cyc/col): per (k-tile j, head) one DR instr:
    sub0 = K-head quadrant weights (zero-padded other 64 partitions)
    sub1 = diag(-240) mask weights vs a 240*lower-triangle constant window
           (diag tiles) or a zero window (off-diag) -> causal mask rides the
           otherwise-wasted second DR slot: -57600 pre-exp = exp -> 0.
  exp on ScalarE merged across both heads ([2,N] strided), out bf16 probs.
  PV in bf16 (plain matmuls, exact causal narrowing, no widening penalty),
  ones-column in V gives softmax denominators as psum row 64.
  normalize: reciprocal(denom row) read straight from PSUM, gpsimd partition
  broadcast, one fused DVE multiply PSUM->ctx_all bf16.
  O-projection in bf16 from ctx_all; full [128, D] output rows -> one
  contiguous DMA per s-tile.
"""
import sys
for _p in ('/opt/trn_rl_repo', '/root/.axon_site/_ro/trn_rl_repo'):
    if _p not in sys.path:
        sys.path.insert(0, _p)

import numpy as np
import ml_dtypes

B, S, D, H = 4, 2048, 1024, 16
DH = 64
N_CORES = 8
HL = H // 2           # local heads per core
DL = HL * DH          # local ctx dims per core

E4NP = ml_dtypes.float8_e4m3
BFNP = ml_dtypes.bfloat16
MV = 240.0            # fp8e4m3 max finite (IEEE variant used by mybir)

# reserved columns in the qt plane: [0,2048) Q, [2048,2176) tri mask const,
# [2176,3072) zeros
QTW = 3072
TRI0 = 2048
ZERO0 = 2560          # off-diag mask windows start here (all zeros)


def build_nc(s=S, d=D, hl=HL, n_cores=N_CORES, reps=1):
    import concourse.bacc as bacc
    import concourse.mybir as mybir
    import concourse.tile as tile
    from concourse.ap import AP

    DT = mybir.dt
    F32 = DT.float32
    F32R = DT.float32r
    BF16 = DT.bfloat16
    FP8 = DT.float8e4
    AFT = mybir.ActivationFunctionType
    DR = mybir.MatmulPerfMode.DoubleRow

    dl = hl * DH
    n_kt = s // 128       # k/s tiles
    n_ch = s // 512       # 512-wide q chunks
    n_dt = d // 128       # d_model tiles
    n_oc = d // 512       # output d chunks
    pairs = hl // 2

    nc = bacc.Bacc("TRN2", target_bir_lowering=False, debug=False,
                   num_devices=n_cores)
    xh8 = nc.declare_dram_parameter("xh8", [d, s], FP8, isOutput=False)
    xl8 = nc.declare_dram_parameter("xl8", [d, s], FP8, isOutput=False)
    wq8 = {}
    wk8 = {}
    for nm in ("A", "B", "C"):
        wq8[nm] = nc.declare_dram_parameter(f"wq8{nm}", [d, dl], FP8, isOutput=False)
        wk8[nm] = nc.declare_dram_parameter(f"wk8{nm}", [d, dl], FP8, isOutput=False)
    wv8 = {}
    for nm in ("A", "B", "C"):
        wv8[nm] = nc.declare_dram_parameter(f"wv8{nm}", [d, dl], FP8, isOutput=False)
    woT = nc.declare_dram_parameter("woT", [dl, d], BF16, isOutput=False)
    dg = nc.declare_dram_parameter("dg", [128, 128], FP8, isOutput=False)
    t240 = nc.declare_dram_parameter("t240", [128, 128], FP8, isOutput=False)
    F16 = DT.float16
    out = nc.declare_dram_parameter("out", [s, d], F16, isOutput=True)

    def dr_rhs(plane_ap, o0, o1, n):
        """[128, 2, n] rhs AP: sub0 at col o0 (Q window), sub1 at col o1
        (mask/zero window) of a [128, QTW] qt plane."""
        base = plane_ap[:, o0:o0 + n]
        return AP(base.tensor, base.offset,
                  [list(base.ap[0]), [o1 - o0, 2], [1, n]])

    with tile.TileContext(nc) as tc:
        with tc.tile_pool(name="persist", bufs=1) as pp, \
             tc.tile_pool(name="stream", bufs=1) as sp, \
             tc.tile_pool(name="psum", bufs=1, space="PSUM") as ps:

            # ---- resident tensors ----
            v_sb = pp.tile([128, n_kt, hl, DH + 1], BF16, name="v_sb")   # V + ones col
            ctx_all = pp.tile([128, pairs, s], BF16, name="ctx_all")     # normalized ctx^T
            qt_all = pp.tile([128, 2, QTW], FP8, name="qt_all")          # Q fp8 + consts
            xh = pp.tile([128, n_dt, s], FP8, name="xh")
            xl = pp.tile([128, n_dt, s], FP8, name="xl")
            kt_dr = pp.tile([128, n_kt, 2, 2, 128], FP8, name="kt_dr")   # K DR weights

            # one-time inits (overlap with the initial xT DMA)
            nc.gpsimd.dma_start(out=qt_all[:, 0, TRI0:TRI0 + 128], in_=t240[:, :])
            nc.gpsimd.dma_start(out=qt_all[:, 1, TRI0:TRI0 + 128], in_=t240[:, :])
            nc.gpsimd.memset(qt_all[:, :, TRI0 + 128:QTW].bitcast(F32), 0.0)
            # K sub0 zero quadrants (evacs only write each head's 64 rows)
            nc.gpsimd.memset(kt_dr[:, :, :, 0, :].bitcast(F32), 0.0)
            # mask weights: sub1 = diag(-240) for every (j, head)
            for j in range(n_kt):
                nc.gpsimd.dma_start(out=kt_dr[:, j, :, 1, :],
                                    in_=dg[:, :].unsqueeze(1).broadcast_to([128, 2, 128]))
            nc.gpsimd.memset(v_sb[:, :, :, DH:DH + 1], 1.0)

            for _rep in range(reps):
              R = f"{_rep}_" if reps > 1 else ""

              # interleave (wv[t], xt[t]) across both HWDGE queues so the
              # t-accumulation can pace with DMA arrivals; xt lands in
              # col-chunk-major order.
              wvs = {}
              for i, nm in enumerate(("A", "B", "C")):
                  wvs[nm] = sp.tile([128, n_dt, dl], FP8, name=f"{R}wv{nm}_sb",
                                    tag=f"wv{nm}")
              for c in range(n_ch):
                  for t in range(n_dt):
                      eng = nc.sync if t % 2 == 0 else nc.gpsimd
                      eng2 = nc.gpsimd if t % 2 == 0 else nc.sync
                      if c == 0:
                          for i, nm in enumerate(("A", "B", "C")):
                              (eng if i % 2 == 0 else eng2).dma_start(
                                  out=wvs[nm][:, t, :],
                                  in_=wv8[nm][128 * t:128 * (t + 1), :])
                      eng.dma_start(out=xh[:, t, 512 * c:512 * (c + 1)],
                                    in_=xh8[128 * t:128 * (t + 1), 512 * c:512 * (c + 1)])
                      eng2.dma_start(out=xl[:, t, 512 * c:512 * (c + 1)],
                                     in_=xl8[128 * t:128 * (t + 1), 512 * c:512 * (c + 1)])
              # ---- V projection (f32r) -> v_sb bf16 ----
              for kt in range(n_kt):
                  if kt % 2 == 0:
                      pvg = ps.tile([128, 2, 512], F32, name=f"{R}pv_{kt}",
                                    tag=("pp", "sg")[(kt // 2) % 2],
                                    bufs=(1 if (kt // 2) % 2 == 0 else 2))
                  pv = pvg[:, kt % 2, :]
                  ks_ = slice(128 * kt, 128 * (kt + 1))
                  i = 0
                  for nm, xsrc in (("A", xh), ("B", xl), ("C", xh)):
                      for t2 in range(n_dt // 2):
                          nc.tensor.matmul(pv, xsrc[:, 2 * t2:2 * t2 + 2, ks_],
                                           wvs[nm][:, 2 * t2:2 * t2 + 2, :],
                                           start=(i == 0),
                                           stop=(i == 3 * (n_dt // 2) - 1),
                                           perf_mode=DR)
                          i += 1
                  nc.vector.tensor_scalar_mul(
                      v_sb[:, kt, :, 0:DH],
                      pv.rearrange("p (h e) -> p h e", e=DH), 2.0 ** -13)

              # ---- per head-pair ----
              for p in range(pairs):
                  pr = p % 2
                  wqs = {}
                  wks = {}
                  for i, nm in enumerate(("A", "B", "C")):
                      wqs[nm] = sp.tile([128, n_dt, 128], FP8,
                                        name=f"{R}wq{nm}_{p}", tag=f"wq{nm}")
                      wks[nm] = sp.tile([128, n_dt, 128], FP8,
                                        name=f"{R}wk{nm}_{p}", tag=f"wk{nm}")
                      wq_r = wq8[nm].rearrange("(t r) m -> r t m", r=128)
                      wk_r = wk8[nm].rearrange("(t r) m -> r t m", r=128)
                      nc.gpsimd.dma_start(out=wqs[nm],
                                          in_=wq_r[:, :, 128 * p:128 * (p + 1)])
                      nc.sync.dma_start(out=wks[nm],
                                        in_=wk_r[:, :, 128 * p:128 * (p + 1)])

                  # Q/K projections (f32r) -> fp8 evacuation (x16)
                  for c4 in range(n_ch):
                      g = ps.tile([128, 2, 512], F32, name=f"{R}pqk_{p}_{c4}",
                                  tag=("sg", "pp")[c4 % 2],
                                  bufs=(2 if c4 % 2 == 0 else 1))
                      psq, psk = g[:, 0, :], g[:, 1, :]
                      cs = slice(512 * c4, 512 * (c4 + 1))
                      prods = (("A", xh), ("B", xl), ("C", xh))
                      for ps_, ws_ in ((psq, wqs), (psk, wks)):
                          i = 0
                          for nm, xsrc in prods:
                              for t2 in range(n_dt // 2):
                                  nc.tensor.matmul(
                                      ps_, ws_[nm][:, 2 * t2:2 * t2 + 2, :],
                                      xsrc[:, 2 * t2:2 * t2 + 2, cs],
                                      start=(i == 0),
                                      stop=(i == 3 * (n_dt // 2) - 1),
                                      perf_mode=DR)
                                  i += 1
                      nc.vector.tensor_scalar_mul(
                          qt_all[:, pr, 512 * c4:512 * (c4 + 1)], psq, 2.0 ** -9)
                      # K quadrant evacuations: head A rows 0:64, head B 64:128
                      ja, jb = 4 * c4, 4 * (c4 + 1)
                      nc.vector.tensor_scalar_mul(
                          kt_dr[0:64, ja:jb, 0, 0, :],
                          psk[0:64, :].rearrange("p (j m) -> p j m", m=128), 2.0 ** -9)
                      nc.vector.tensor_scalar_mul(
                          kt_dr[64:128, ja:jb, 1, 0, :],
                          psk[64:128, :].rearrange("p (j m) -> p j m", m=128), 2.0 ** -9)

                  # ---- attention over chunks ----
                  qplane = qt_all[:, pr, :]
                  for c4 in range(n_ch):
                      q0 = 512 * c4
                      ctxA = ps.tile([DH + 1, 512], F32, name=f"{R}cA_{p}_{c4}", tag="ctxA")
                      ctxB = ps.tile([DH + 1, 512], F32, name=f"{R}cB_{p}_{c4}", tag="ctxB")
                      nj = 4 * c4 + 4
                      pending = None
                      for j in range(nj):
                          m = j - 4 * c4
                          n0 = 128 * m if m >= 0 else 0
                          w = 512 - n0
                          sg = ps.tile([128, 2, 512], F32, name=f"{R}sg_{p}_{c4}_{j}",
                                       tag="sg", bufs=2)
                          o1 = TRI0 if m >= 0 else ZERO0
                          rhs = dr_rhs(qplane, q0 + n0, o1, w)
                          for h in (0, 1):
                              nc.tensor.matmul(sg[:, h, n0:512],
                                               kt_dr[:, j, h, :, :], rhs,
                                               start=True, stop=True, perf_mode=DR)
                          pt = sp.tile([128, 2, 512], BF16, name=f"{R}pt_{p}_{c4}_{j}",
                                       tag="pt", bufs=4)
                          nc.scalar.activation(out=pt[:, :, n0:512],
                                               in_=sg[:, :, n0:512],
                                               func=AFT.Exp, scale=0.125 / 256.0)
                          if pending is not None:
                              _emit_pv(nc, v_sb, ctxA, ctxB, p, pending, nj)
                          pending = (j, pt, n0)
                      _emit_pv(nc, v_sb, ctxA, ctxB, p, pending, nj)

                      # normalize (baseline flow: the custom DVE ops only
                      # handle full-tile APs on HW): evacuate ctx + denom to
                      # SBUF, reciprocal, partition broadcast, in-place mult
                      for head, cpsum in ((0, ctxA), (1, ctxB)):
                          r_i = 2 * c4 + head
                          hs = slice(64 * head, 64 * head + 64)
                          nc.vector.tensor_copy(
                              out=ctx_all[hs, p, q0:q0 + 512],
                              in_=cpsum[0:DH, :])
                          dn1 = sp.tile([1, 512], F32, name=f"{R}dn_{p}_{r_i}",
                                        tag="dn", bufs=2)
                          nc.vector.tensor_copy(out=dn1, in_=cpsum[DH:DH + 1, :])
                          rb1 = sp.tile([1, 512], F32, name=f"{R}rc_{p}_{r_i}",
                                        tag="rc", bufs=2)
                          nc.vector.reciprocal_approx_fast(out=rb1, in_=dn1)
                          rb = sp.tile([128, 512], F32, name=f"{R}rb_{p}_{r_i}",
                                       tag="rb", bufs=2)
                          nc.gpsimd.partition_broadcast(rb, rb1)
                          cslice = ctx_all[hs, p, q0:q0 + 512]
                          nc.vector.tensor_mul(cslice, cslice, rb[hs, :])

              # ---- O projection (bf16, partial: contracts local 512 dims) ----
              wo_sb = sp.tile([128, pairs, d], BF16, name=f"{R}wo_sb", tag="wvo")
              for ct in range(pairs):
                  nc.sync.dma_start(out=wo_sb[:, ct, :],
                                    in_=woT[128 * ct:128 * (ct + 1), :])
              for st_i in range(n_kt):
                  ot = sp.tile([128, d], F16, name=f"{R}ot_{st_i}",
                               tag="ot", bufs=3)
                  if st_i % 2 == 0:
                      og = ps.tile([128, 2, 512], F32, name=f"{R}po_{st_i}",
                                   tag=("pp", "sg")[(st_i // 2) % 2],
                                   bufs=(1 if (st_i // 2) % 2 == 0 else 2))
                  for oc in range(n_oc):
                      pso = og[:, (2 * st_i + oc) % 2, :]
                      for ct in range(pairs):
                          nc.tensor.matmul(pso,
                                           ctx_all[:, ct, 128 * st_i:128 * (st_i + 1)],
                                           wo_sb[:, ct, 512 * oc:512 * (oc + 1)],
                                           start=(ct == 0), stop=(ct == pairs - 1))
                      nc.vector.tensor_copy(out=ot[:, 512 * oc:512 * (oc + 1)],
                                            in_=pso)
                  oeng = nc.sync if st_i % 2 == 0 else nc.gpsimd
                  oeng.dma_start(out=out[128 * st_i:128 * (st_i + 1), :], in_=ot)

    nc.compile()
    return nc


def _emit_pv(nc, v_sb, ctxA, ctxB, p, pending, nj):
    j, pt, n0 = pending
    start = (j == 0)
    stop = (j == nj - 1)
    nc.tensor.matmul(ctxA[:, n0:512], v_sb[:, j, 2 * p, :], pt[:, 0, n0:512],
                     start=start, stop=stop)
    nc.tensor.matmul(ctxB[:, n0:512], v_sb[:, j, 2 * p + 1, :], pt[:, 1, n0:512],
                     start=start, stop=stop)


def make_dg():
    m = np.zeros((128, 128), np.float32)
    np.fill_diagonal(m, -MV)
    return m.astype(E4NP)


def make_t240():
    k = np.arange(128)[:, None]
    q = np.arange(128)[None, :]
    return (MV * (q < k)).astype(np.float32).astype(E4NP)


def shard_inputs(in_features, q_weight, k_weight, v_weight, o_weight):
    """-> list of 8 per-core input dicts."""
    dgm = make_dg()
    t240 = make_t240()
    maps = []
    for c in range(N_CORES):
        b, g = divmod(c, 2)
        hs = slice(DL * g, DL * (g + 1))   # local head dims in the full D
        xt = np.ascontiguousarray(in_features[b].T)
        xh = (xt * 16.0).astype(E4NP)
        xl = ((xt * 16.0 - xh.astype(np.float32)) * 16.0).astype(E4NP)
        m = {
            "xh8": xh,
            "xl8": xl,
            "woT": np.ascontiguousarray(o_weight[:, hs].T.astype(BFNP)),
            "dg": dgm,
            "t240": t240,
        }
        for wname, w in (("wq8", q_weight), ("wk8", k_weight), ("wv8", v_weight)):
            wt = np.ascontiguousarray(w[hs, :].T).astype(np.float32)
            A = (wt * 512.0).astype(E4NP)
            Bm = (wt * 32.0).astype(E4NP)
            C = (wt * 512.0 - A.astype(np.float32)).astype(E4NP)
            m[wname + "A"] = A
            m[wname + "B"] = Bm
            m[wname + "C"] = C
        maps.append(m)
    return maps


def gather_output(results):
    """results: list of 8 dicts with 'out' [S, D] partials -> [B, S, D]."""
    return np.stack([results[2 * b]["out"].astype(np.float32)
                     + results[2 * b + 1]["out"].astype(np.float32)
                     for b in range(B)])


_nc_cache = {}


def kernel(in_features, q_weight, k_weight, v_weight, o_weight):
    from concourse.bass_utils import run_bass_kernel_spmd
    if "nc" not in _nc_cache:
        _nc_cache["nc"] = build_nc()
    nc = _nc_cache["nc"]
    in_maps = shard_inputs(np.asarray(in_features, dtype=np.float32),
                           np.asarray(q_weight, dtype=np.float32),
                           np.asarray(k_weight, dtype=np.float32),
                           np.asarray(v_weight, dtype=np.float32),
                           np.asarray(o_weight, dtype=np.float32))
    res = run_bass_kernel_spmd(nc, in_maps, core_ids=list(range(N_CORES)))
    return gather_output(res.results)


# revision 9
# speedup vs baseline: 1.0325x; 1.0325x over previous
"""Multi-head causal self-attention (B=4, S=2048, D=1024, H=16) on 8 NeuronCores.

Sharding: core c handles batch b=c//2 and heads [8*(c%2), 8*(c%2)+8) (tensor
parallel over heads x data parallel over batch). Each core computes its 8
heads' Q/K/V projections, causal attention, and a partial O-projection
(contracting only its 512 ctx dims). Host sums the two partial outputs per
batch.

Per-core math / precision plan:
  Q/K/V projections in float32r (full PE rate, ~exact).
  Q/K evacuated to fp8e4m3 (x16); scores via fp8 DoubleRow matmuls (0.5
  cyc/col): per (k-tile j, head) one DR instr:
    sub0 = K-head quadrant weights (zero-padded other 64 partitions)
    sub1 = diag(-240) mask weights vs a 240*lower-triangle constant window
           (diag tiles) or a zero window (off-diag) -> causal mask rides the
           otherwise-wasted second DR slot: -57600 pre-exp = exp -> 0.
  exp on ScalarE merged across both heads ([2,N] strided), out bf16 probs.
  PV in bf16 (plain matmuls, exact causal narrowing, no widening penalty),
  ones-column in V gives softmax denominators as psum row 64.
  normalize: reciprocal(denom row) read straight from PSUM, gpsimd partition
  broadcast, one fused DVE multiply PSUM->ctx_all bf16.
  O-projection in bf16 from ctx_all; full [128, D] output rows -> one
  contiguous DMA per s-tile.
"""
import sys
for _p in ('/opt/trn_rl_repo', '/root/.axon_site/_ro/trn_rl_repo'):
    if _p not in sys.path:
        sys.path.insert(0, _p)

import numpy as np
import ml_dtypes

B, S, D, H = 4, 2048, 1024, 16
DH = 64
N_CORES = 8
HL = H // 2           # local heads per core
DL = HL * DH          # local ctx dims per core

E4NP = ml_dtypes.float8_e4m3
BFNP = ml_dtypes.bfloat16
MV = 240.0            # fp8e4m3 max finite (IEEE variant used by mybir)

# reserved columns in the qt plane: [0,2048) Q, [2048,2176) tri mask const,
# [2176,3072) zeros
QTW = 3072
TRI0 = 2048
ZERO0 = 2560          # off-diag mask windows start here (all zeros)


def build_nc(s=S, d=D, hl=HL, n_cores=N_CORES, reps=1):
    import concourse.bacc as bacc
    import concourse.mybir as mybir
    import concourse.tile as tile
    from concourse.ap import AP

    DT = mybir.dt
    F32 = DT.float32
    F32R = DT.float32r
    BF16 = DT.bfloat16
    FP8 = DT.float8e4
    AFT = mybir.ActivationFunctionType
    DR = mybir.MatmulPerfMode.DoubleRow

    dl = hl * DH
    n_kt = s // 128       # k/s tiles
    n_ch = s // 512       # 512-wide q chunks
    n_dt = d // 128       # d_model tiles
    n_oc = d // 512       # output d chunks
    pairs = hl // 2

    nc = bacc.Bacc("TRN2", target_bir_lowering=False, debug=False,
                   num_devices=n_cores)
    xh8 = nc.declare_dram_parameter("xh8", [d, s], FP8, isOutput=False)
    xl8 = nc.declare_dram_parameter("xl8", [d, s], FP8, isOutput=False)
    wq8 = {}
    wk8 = {}
    for nm in ("A", "B", "C"):
        wq8[nm] = nc.declare_dram_parameter(f"wq8{nm}", [dl // 128, 128, n_dt * 128],
                                            FP8, isOutput=False)
        wk8[nm] = nc.declare_dram_parameter(f"wk8{nm}", [dl // 128, 128, n_dt * 128],
                                            FP8, isOutput=False)
    wv8 = {}
    for nm in ("A", "B", "C"):
        wv8[nm] = nc.declare_dram_parameter(f"wv8{nm}", [d, dl], FP8, isOutput=False)
    woT = nc.declare_dram_parameter("woT", [dl, d], BF16, isOutput=False)
    dg = nc.declare_dram_parameter("dg", [128, 128], FP8, isOutput=False)
    t240 = nc.declare_dram_parameter("t240", [128, 128], FP8, isOutput=False)
    F16 = DT.float16
    out = nc.declare_dram_parameter("out", [s, d], F16, isOutput=True)

    def dr_rhs(plane_ap, o0, o1, n):
        """[128, 2, n] rhs AP: sub0 at col o0 (Q window), sub1 at col o1
        (mask/zero window) of a [128, QTW] qt plane."""
        base = plane_ap[:, o0:o0 + n]
        return AP(base.tensor, base.offset,
                  [list(base.ap[0]), [o1 - o0, 2], [1, n]])

    with tile.TileContext(nc) as tc:
        with tc.tile_pool(name="persist", bufs=1) as pp, \
             tc.tile_pool(name="stream", bufs=1) as sp, \
             tc.tile_pool(name="psum", bufs=1, space="PSUM") as ps:

            # ---- resident tensors ----
            v_sb = pp.tile([128, n_kt, hl, DH + 1], BF16, name="v_sb")   # V + ones col
            ctx_all = pp.tile([128, pairs, s], BF16, name="ctx_all")     # normalized ctx^T
            qt_all = pp.tile([128, 2, QTW], FP8, name="qt_all")          # Q fp8 + consts
            xh = pp.tile([128, n_dt, s], FP8, name="xh")
            xl = pp.tile([128, n_dt, s], FP8, name="xl")
            kt_dr = pp.tile([128, n_kt, 2, 2, 128], FP8, name="kt_dr")   # K DR weights

            # one-time inits (overlap with the initial xT DMA)
            nc.gpsimd.dma_start(out=qt_all[:, 0, TRI0:TRI0 + 128], in_=t240[:, :])
            nc.gpsimd.dma_start(out=qt_all[:, 1, TRI0:TRI0 + 128], in_=t240[:, :])
            nc.gpsimd.memset(qt_all[:, :, TRI0 + 128:QTW].bitcast(F32), 0.0)
            # K sub0 zero quadrants (evacs only write each head's 64 rows)
            nc.gpsimd.memset(kt_dr[:, :, :, 0, :].bitcast(F32), 0.0)
            # mask weights: sub1 = diag(-240) for every (j, head)
            for j in range(n_kt):
                nc.gpsimd.dma_start(out=kt_dr[:, j, :, 1, :],
                                    in_=dg[:, :].unsqueeze(1).broadcast_to([128, 2, 128]))
            nc.gpsimd.memset(v_sb[:, :, :, DH:DH + 1], 1.0)

            for _rep in range(reps):
              R = f"{_rep}_" if reps > 1 else ""

              # interleave (wv[t], xt[t]) across both HWDGE queues so the
              # t-accumulation can pace with DMA arrivals; xt lands in
              # col-chunk-major order.
              wvs = {}
              for i, nm in enumerate(("A", "B", "C")):
                  wvs[nm] = sp.tile([128, n_dt, dl], FP8, name=f"{R}wv{nm}_sb",
                                    tag=f"wv{nm}")
              for i, nm in enumerate(("A", "B", "C")):
                  (nc.sync if i % 2 == 0 else nc.gpsimd).dma_start(
                      out=wvs[nm],
                      in_=wv8[nm].rearrange("(t r) m -> r t m", r=128))
              for t in range(n_dt):
                  eng = nc.sync if t % 2 == 0 else nc.gpsimd
                  eng2 = nc.gpsimd if t % 2 == 0 else nc.sync
                  eng.dma_start(out=xh[:, t, :], in_=xh8[128 * t:128 * (t + 1), :])
                  eng2.dma_start(out=xl[:, t, :], in_=xl8[128 * t:128 * (t + 1), :])
              # ---- V projection (f32r) -> v_sb bf16 ----
              for kt in range(n_kt):
                  if kt % 2 == 0:
                      pvg = ps.tile([128, 2, 512], F32, name=f"{R}pv_{kt}",
                                    tag=("pp", "sg")[(kt // 2) % 2],
                                    bufs=(1 if (kt // 2) % 2 == 0 else 2))
                  pv = pvg[:, kt % 2, :]
                  ks_ = slice(128 * kt, 128 * (kt + 1))
                  i = 0
                  for nm, xsrc in (("A", xh), ("B", xl), ("C", xh)):
                      for t2 in range(n_dt // 2):
                          nc.tensor.matmul(pv, xsrc[:, 2 * t2:2 * t2 + 2, ks_],
                                           wvs[nm][:, 2 * t2:2 * t2 + 2, :],
                                           start=(i == 0),
                                           stop=(i == 3 * (n_dt // 2) - 1),
                                           perf_mode=DR)
                          i += 1
                  nc.vector.tensor_scalar_mul(
                      v_sb[:, kt, :, 0:DH],
                      pv.rearrange("p (h e) -> p h e", e=DH), 2.0 ** -13)

              # ---- per head-pair ----
              for p in range(pairs):
                  pr = p % 2
                  wqs = {}
                  wks = {}
                  for i, nm in enumerate(("A", "B", "C")):
                      wqs[nm] = sp.tile([128, n_dt, 128], FP8,
                                        name=f"{R}wq{nm}_{p}", tag=f"wq{nm}")
                      wks[nm] = sp.tile([128, n_dt, 128], FP8,
                                        name=f"{R}wk{nm}_{p}", tag=f"wk{nm}")
                      nc.gpsimd.dma_start(
                          out=wqs[nm],
                          in_=wq8[nm][p].rearrange("r (t m) -> r t m", m=128))
                      nc.sync.dma_start(
                          out=wks[nm],
                          in_=wk8[nm][p].rearrange("r (t m) -> r t m", m=128))

                  # Q/K projections (f32r) -> fp8 evacuation (x16)
                  for c4 in range(n_ch):
                      g = ps.tile([128, 2, 512], F32, name=f"{R}pqk_{p}_{c4}",
                                  tag=("sg", "pp")[c4 % 2],
                                  bufs=(2 if c4 % 2 == 0 else 1))
                      psq, psk = g[:, 0, :], g[:, 1, :]
                      cs = slice(512 * c4, 512 * (c4 + 1))
                      prods = (("A", xh), ("B", xl), ("C", xh))
                      for ps_, ws_ in ((psq, wqs), (psk, wks)):
                          i = 0
                          for nm, xsrc in prods:
                              for t2 in range(n_dt // 2):
                                  nc.tensor.matmul(
                                      ps_, ws_[nm][:, 2 * t2:2 * t2 + 2, :],
                                      xsrc[:, 2 * t2:2 * t2 + 2, cs],
                                      start=(i == 0),
                                      stop=(i == 3 * (n_dt // 2) - 1),
                                      perf_mode=DR)
                                  i += 1
                      nc.vector.tensor_scalar_mul(
                          qt_all[:, pr, 512 * c4:512 * (c4 + 1)], psq, 2.0 ** -9)
                      # K quadrant evacuations: head A rows 0:64, head B 64:128
                      ja, jb = 4 * c4, 4 * (c4 + 1)
                      nc.vector.tensor_scalar_mul(
                          kt_dr[0:64, ja:jb, 0, 0, :],
                          psk[0:64, :].rearrange("p (j m) -> p j m", m=128), 2.0 ** -9)
                      nc.vector.tensor_scalar_mul(
                          kt_dr[64:128, ja:jb, 1, 0, :],
                          psk[64:128, :].rearrange("p (j m) -> p j m", m=128), 2.0 ** -9)

                  # ---- attention over chunks ----
                  qplane = qt_all[:, pr, :]
                  for c4 in range(n_ch):
                      q0 = 512 * c4
                      ctxA = ps.tile([DH + 1, 512], F32, name=f"{R}cA_{p}_{c4}", tag="ctxA")
                      ctxB = ps.tile([DH + 1, 512], F32, name=f"{R}cB_{p}_{c4}", tag="ctxB")
                      nj = 4 * c4 + 4
                      pending = None
                      for j in range(nj):
                          m = j - 4 * c4
                          n0 = 128 * m if m >= 0 else 0
                          w = 512 - n0
                          sg = ps.tile([128, 2, 512], F32, name=f"{R}sg_{p}_{c4}_{j}",
                                       tag="sg", bufs=2)
                          o1 = TRI0 if m >= 0 else ZERO0
                          rhs = dr_rhs(qplane, q0 + n0, o1, w)
                          for h in (0, 1):
                              nc.tensor.matmul(sg[:, h, n0:512],
                                               kt_dr[:, j, h, :, :], rhs,
                                               start=True, stop=True, perf_mode=DR)
                          pt = sp.tile([128, 2, 512], BF16, name=f"{R}pt_{p}_{c4}_{j}",
                                       tag="pt", bufs=4)
                          nc.scalar.activation(out=pt[:, :, n0:512],
                                               in_=sg[:, :, n0:512],
                                               func=AFT.Exp, scale=0.125 / 256.0)
                          if pending is not None:
                              _emit_pv(nc, v_sb, ctxA, ctxB, p, pending, nj)
                          pending = (j, pt, n0)
                      _emit_pv(nc, v_sb, ctxA, ctxB, p, pending, nj)

                      # normalize (baseline flow: the custom DVE ops only
                      # handle full-tile APs on HW): evacuate ctx + denom to
                      # SBUF, reciprocal, partition broadcast, in-place mult
                      for head, cpsum in ((0, ctxA), (1, ctxB)):
                          r_i = 2 * c4 + head
                          hs = slice(64 * head, 64 * head + 64)
                          nc.vector.tensor_copy(
                              out=ctx_all[hs, p, q0:q0 + 512],
                              in_=cpsum[0:DH, :])
                          dn1 = sp.tile([1, 512], F32, name=f"{R}dn_{p}_{r_i}",
                                        tag="dn", bufs=2)
                          nc.vector.tensor_copy(out=dn1, in_=cpsum[DH:DH + 1, :])
                          rb1 = sp.tile([1, 512], F32, name=f"{R}rc_{p}_{r_i}",
                                        tag="rc", bufs=2)
                          nc.vector.reciprocal_approx_fast(out=rb1, in_=dn1)
                          rb = sp.tile([128, 512], F32, name=f"{R}rb_{p}_{r_i}",
                                       tag="rb", bufs=2)
                          nc.gpsimd.partition_broadcast(rb, rb1)
                          cslice = ctx_all[hs, p, q0:q0 + 512]
                          nc.vector.tensor_mul(cslice, cslice, rb[hs, :])

              # ---- O projection (bf16, partial: contracts local 512 dims) ----
              wo_sb = sp.tile([128, pairs, d], BF16, name=f"{R}wo_sb", tag="wvo")
              for ct in range(pairs):
                  nc.sync.dma_start(out=wo_sb[:, ct, :],
                                    in_=woT[128 * ct:128 * (ct + 1), :])
              for st_i in range(n_kt):
                  ot = sp.tile([128, d], F16, name=f"{R}ot_{st_i}",
                               tag="ot", bufs=3)
                  if st_i % 2 == 0:
                      og = ps.tile([128, 2, 512], F32, name=f"{R}po_{st_i}",
                                   tag=("pp", "sg")[(st_i // 2) % 2],
                                   bufs=(1 if (st_i // 2) % 2 == 0 else 2))
                  for oc in range(n_oc):
                      pso = og[:, (2 * st_i + oc) % 2, :]
                      for ct in range(pairs):
                          nc.tensor.matmul(pso,
                                           ctx_all[:, ct, 128 * st_i:128 * (st_i + 1)],
                                           wo_sb[:, ct, 512 * oc:512 * (oc + 1)],
                                           start=(ct == 0), stop=(ct == pairs - 1))
                      nc.vector.tensor_copy(out=ot[:, 512 * oc:512 * (oc + 1)],
                                            in_=pso)
                  oeng = nc.sync if st_i % 2 == 0 else nc.gpsimd
                  oeng.dma_start(out=out[128 * st_i:128 * (st_i + 1), :], in_=ot)

    nc.compile()
    return nc


def _emit_pv(nc, v_sb, ctxA, ctxB, p, pending, nj):
    j, pt, n0 = pending
    start = (j == 0)
    stop = (j == nj - 1)
    nc.tensor.matmul(ctxA[:, n0:512], v_sb[:, j, 2 * p, :], pt[:, 0, n0:512],
                     start=start, stop=stop)
    nc.tensor.matmul(ctxB[:, n0:512], v_sb[:, j, 2 * p + 1, :], pt[:, 1, n0:512],
                     start=start, stop=stop)


def make_dg():
    m = np.zeros((128, 128), np.float32)
    np.fill_diagonal(m, -MV)
    return m.astype(E4NP)


def make_t240():
    k = np.arange(128)[:, None]
    q = np.arange(128)[None, :]
    return (MV * (q < k)).astype(np.float32).astype(E4NP)


def shard_inputs(in_features, q_weight, k_weight, v_weight, o_weight):
    """-> list of 8 per-core input dicts."""
    dgm = make_dg()
    t240 = make_t240()
    maps = []
    for c in range(N_CORES):
        b, g = divmod(c, 2)
        hs = slice(DL * g, DL * (g + 1))   # local head dims in the full D
        xt = np.ascontiguousarray(in_features[b].T)
        xh = (xt * 16.0).astype(E4NP)
        xl = ((xt * 16.0 - xh.astype(np.float32)) * 16.0).astype(E4NP)
        m = {
            "xh8": xh,
            "xl8": xl,
            "woT": np.ascontiguousarray(o_weight[:, hs].T.astype(BFNP)),
            "dg": dgm,
            "t240": t240,
        }
        for wname, w in (("wq8", q_weight), ("wk8", k_weight), ("wv8", v_weight)):
            wt = np.ascontiguousarray(w[hs, :].T).astype(np.float32)  # [D, 512]
            A = (wt * 512.0).astype(E4NP)
            Bm = (wt * 32.0).astype(E4NP)
            C = (wt * 512.0 - A.astype(np.float32)).astype(E4NP)
            if wname in ("wq8", "wk8"):
                # -> [pairs, 128 rows, n_dt*128]: SBUF layout, contiguous per pair
                def tile_w(a):
                    # a: [D, 512] -> pair p slice cols 128p:128p+128,
                    # rows (t,r) -> [p, r, t*128+m]
                    a4 = a.reshape(D // 128, 128, 4, 128)      # [t, r, p, m]
                    return np.ascontiguousarray(
                        a4.transpose(2, 1, 0, 3).reshape(4, 128, -1))
                m[wname + "A"] = tile_w(A)
                m[wname + "B"] = tile_w(Bm)
                m[wname + "C"] = tile_w(C)
            else:
                m[wname + "A"] = A
                m[wname + "B"] = Bm
                m[wname + "C"] = C
        maps.append(m)
    return maps


def gather_output(results):
    """results: list of 8 dicts with 'out' [S, D] partials -> [B, S, D]."""
    return np.stack([results[2 * b]["out"].astype(np.float32)
                     + results[2 * b + 1]["out"].astype(np.float32)
                     for b in range(B)])


_nc_cache = {}


def kernel(in_features, q_weight, k_weight, v_weight, o_weight):
    from concourse.bass_utils import run_bass_kernel_spmd
    if "nc" not in _nc_cache:
        _nc_cache["nc"] = build_nc()
    nc = _nc_cache["nc"]
    in_maps = shard_inputs(np.asarray(in_features, dtype=np.float32),
                           np.asarray(q_weight, dtype=np.float32),
                           np.asarray(k_weight, dtype=np.float32),
                           np.asarray(v_weight, dtype=np.float32),
                           np.asarray(o_weight, dtype=np.float32))
    res = run_bass_kernel_spmd(nc, in_maps, core_ids=list(range(N_CORES)))
    return gather_output(res.results)


# revision 10
# speedup vs baseline: 1.3397x; 1.2975x over previous
"""Multi-head causal self-attention (B=4, S=2048, D=1024, H=16) on 8 NeuronCores.

Sharding: core c handles batch b=c//2 and heads [8*(c%2), 8*(c%2)+8) (tensor
parallel over heads x data parallel over batch). Each core computes its 8
heads' Q/K/V projections, causal attention, and a partial O-projection
(contracting only its 512 ctx dims). Host sums the two partial outputs per
batch.

Per-core math / precision plan:
  Q/K/V projections in float32r (full PE rate, ~exact).
  Q/K evacuated to fp8e4m3 (x16); scores via fp8 DoubleRow matmuls (0.5
  cyc/col): per (k-tile j, head) one DR instr:
    sub0 = K-head quadrant weights (zero-padded other 64 partitions)
    sub1 = diag(-240) mask weights vs a 240*lower-triangle constant window
           (diag tiles) or a zero window (off-diag) -> causal mask rides the
           otherwise-wasted second DR slot: -57600 pre-exp = exp -> 0.
  exp on ScalarE merged across both heads ([2,N] strided), out bf16 probs.
  PV in bf16 (plain matmuls, exact causal narrowing, no widening penalty),
  ones-column in V gives softmax denominators as psum row 64.
  normalize: reciprocal(denom row) read straight from PSUM, gpsimd partition
  broadcast, one fused DVE multiply PSUM->ctx_all bf16.
  O-projection in bf16 from ctx_all; full [128, D] output rows -> one
  contiguous DMA per s-tile.
"""
import sys
for _p in ('/opt/trn_rl_repo', '/root/.axon_site/_ro/trn_rl_repo'):
    if _p not in sys.path:
        sys.path.insert(0, _p)

import numpy as np
import ml_dtypes

B, S, D, H = 4, 2048, 1024, 16
QKV_MODE = "f32r"   # "f32r" | "dr3": projection math mode
DH = 64
N_CORES = 8
HL = H // 2           # local heads per core
DL = HL * DH          # local ctx dims per core

E4NP = ml_dtypes.float8_e4m3
BFNP = ml_dtypes.bfloat16
MV = 240.0            # fp8e4m3 max finite (IEEE variant used by mybir)

# reserved columns in the qt plane: [0,2048) Q, [2048,2176) tri mask const,
# [2176,3072) zeros
QTW = 3072
TRI0 = 2048
ZERO0 = 2560          # off-diag mask windows start here (all zeros)


def build_nc(s=S, d=D, hl=HL, n_cores=N_CORES, reps=1):
    import concourse.bacc as bacc
    import concourse.mybir as mybir
    import concourse.tile as tile
    from concourse.ap import AP

    DT = mybir.dt
    F32 = DT.float32
    F32R = DT.float32r
    BF16 = DT.bfloat16
    FP8 = DT.float8e4
    AFT = mybir.ActivationFunctionType
    DR = mybir.MatmulPerfMode.DoubleRow

    dl = hl * DH
    n_kt = s // 128       # k/s tiles
    n_ch = s // 512       # 512-wide q chunks
    n_dt = d // 128       # d_model tiles
    n_oc = d // 512       # output d chunks
    pairs = hl // 2

    nc = bacc.Bacc("TRN2", target_bir_lowering=False, debug=False,
                   num_devices=n_cores)
    if QKV_MODE == "dr3":
        xh8 = nc.declare_dram_parameter("xh8", [d, s], FP8, isOutput=False)
        xl8 = nc.declare_dram_parameter("xl8", [d, s], FP8, isOutput=False)
        wq8 = {}
        wk8 = {}
        wv8 = {}
        for nm in ("A", "B", "C"):
            wq8[nm] = nc.declare_dram_parameter(
                f"wq8{nm}", [dl // 128, 128, n_dt * 128], FP8, isOutput=False)
            wk8[nm] = nc.declare_dram_parameter(
                f"wk8{nm}", [dl // 128, 128, n_dt * 128], FP8, isOutput=False)
            wv8[nm] = nc.declare_dram_parameter(
                f"wv8{nm}", [d, dl], FP8, isOutput=False)
    else:
        xT = nc.declare_dram_parameter("xT", [d, s], F32R, isOutput=False)
        wqT = nc.declare_dram_parameter("wqT", [d, dl], F32R, isOutput=False)
        wkT = nc.declare_dram_parameter("wkT", [d, dl], F32R, isOutput=False)
        wvT = nc.declare_dram_parameter("wvT", [d, dl], F32R, isOutput=False)
    woT = nc.declare_dram_parameter("woT", [dl, d], BF16, isOutput=False)
    dg = nc.declare_dram_parameter("dg", [128, 128], FP8, isOutput=False)
    t240 = nc.declare_dram_parameter("t240", [128, 128], FP8, isOutput=False)
    F16 = DT.float16
    out = nc.declare_dram_parameter("out", [s, d], F16, isOutput=True)

    def dr_rhs(plane_ap, o0, o1, n):
        """[128, 2, n] rhs AP: sub0 at col o0 (Q window), sub1 at col o1
        (mask/zero window) of a [128, QTW] qt plane."""
        base = plane_ap[:, o0:o0 + n]
        return AP(base.tensor, base.offset,
                  [list(base.ap[0]), [o1 - o0, 2], [1, n]])

    with tile.TileContext(nc) as tc:
        with tc.tile_pool(name="persist", bufs=1) as pp, \
             tc.tile_pool(name="stream", bufs=1) as sp, \
             tc.tile_pool(name="psum", bufs=1, space="PSUM") as ps:

            # ---- resident tensors ----
            v_sb = pp.tile([128, n_kt, hl, DH + 1], BF16, name="v_sb")   # V + ones col
            ctx_all = pp.tile([128, pairs, s], BF16, name="ctx_all")     # normalized ctx^T
            qt_all = pp.tile([128, 2, QTW], FP8, name="qt_all")          # Q fp8 + consts
            if QKV_MODE == "dr3":
                xh = pp.tile([128, n_dt, s], FP8, name="xh")
                xl = pp.tile([128, n_dt, s], FP8, name="xl")
            else:
                xt = pp.tile([128, n_dt, s], F32R, name="xt")
            kt_dr = pp.tile([128, n_kt, 2, 2, 128], FP8, name="kt_dr")   # K DR weights

            # one-time inits (overlap with the initial xT DMA)
            nc.gpsimd.dma_start(out=qt_all[:, 0, TRI0:TRI0 + 128], in_=t240[:, :])
            nc.gpsimd.dma_start(out=qt_all[:, 1, TRI0:TRI0 + 128], in_=t240[:, :])
            nc.gpsimd.memset(qt_all[:, :, TRI0 + 128:QTW].bitcast(F32), 0.0)
            # K sub0 zero quadrants (evacs only write each head's 64 rows)
            nc.gpsimd.memset(kt_dr[:, :, :, 0, :].bitcast(F32), 0.0)
            # mask weights: sub1 = diag(-240) for every (j, head)
            for j in range(n_kt):
                nc.gpsimd.dma_start(out=kt_dr[:, j, :, 1, :],
                                    in_=dg[:, :].unsqueeze(1).broadcast_to([128, 2, 128]))
            nc.gpsimd.memset(v_sb[:, :, :, DH:DH + 1], 1.0)

            for _rep in range(reps):
              R = f"{_rep}_" if reps > 1 else ""

              # interleave (wv[t], xt[t]) across both HWDGE queues so the
              # t-accumulation can pace with DMA arrivals; xt lands in
              # col-chunk-major order.
              if QKV_MODE == "dr3":
                  wvs = {}
                  for i, nm in enumerate(("A", "B", "C")):
                      wvs[nm] = sp.tile([128, n_dt, dl], FP8, name=f"{R}wv{nm}_sb",
                                        tag=f"wv{nm}")
                      (nc.sync if i % 2 == 0 else nc.gpsimd).dma_start(
                          out=wvs[nm],
                          in_=wv8[nm].rearrange("(t r) m -> r t m", r=128))
                  for t in range(n_dt):
                      eng = nc.sync if t % 2 == 0 else nc.gpsimd
                      eng2 = nc.gpsimd if t % 2 == 0 else nc.sync
                      eng.dma_start(out=xh[:, t, :], in_=xh8[128 * t:128 * (t + 1), :])
                      eng2.dma_start(out=xl[:, t, :], in_=xl8[128 * t:128 * (t + 1), :])
              else:
                  wv_sb = sp.tile([128, n_dt, dl], F32R, name=f"{R}wv_sb", tag="wvA")
                  for c in range(n_ch):
                      for t in range(n_dt):
                          eng = nc.sync if t % 2 == 0 else nc.gpsimd
                          if c == 0:
                              eng.dma_start(out=wv_sb[:, t, :],
                                            in_=wvT[128 * t:128 * (t + 1), :])
                          eng.dma_start(out=xt[:, t, 512 * c:512 * (c + 1)],
                                        in_=xT[128 * t:128 * (t + 1),
                                               512 * c:512 * (c + 1)])
              # ---- V projection (f32r) -> v_sb bf16 ----
              for kt in range(n_kt):
                  if kt % 2 == 0:
                      pvg = ps.tile([128, 2, 512], F32, name=f"{R}pv_{kt}",
                                    tag=("pp", "sg")[(kt // 2) % 2],
                                    bufs=(1 if (kt // 2) % 2 == 0 else 2))
                  pv = pvg[:, kt % 2, :]
                  ks_ = slice(128 * kt, 128 * (kt + 1))
                  if QKV_MODE == "dr3":
                      i = 0
                      for nm, xsrc in (("A", xh), ("B", xl), ("C", xh)):
                          for t2 in range(n_dt // 2):
                              nc.tensor.matmul(pv, xsrc[:, 2 * t2:2 * t2 + 2, ks_],
                                               wvs[nm][:, 2 * t2:2 * t2 + 2, :],
                                               start=(i == 0),
                                               stop=(i == 3 * (n_dt // 2) - 1),
                                               perf_mode=DR)
                              i += 1
                      nc.vector.tensor_scalar_mul(
                          v_sb[:, kt, :, 0:DH],
                          pv.rearrange("p (h e) -> p h e", e=DH), 2.0 ** -13)
                  else:
                      for t in range(n_dt):
                          nc.tensor.matmul(pv, xt[:, t, ks_], wv_sb[:, t, :],
                                           start=(t == 0), stop=(t == n_dt - 1))
                      nc.vector.tensor_copy(
                          out=v_sb[:, kt, :, 0:DH],
                          in_=pv.rearrange("p (h e) -> p h e", e=DH))

              # ---- per head-pair ----
              for p in range(pairs):
                  pr = p % 2
                  if QKV_MODE == "dr3":
                      wqs = {}
                      wks = {}
                      for i, nm in enumerate(("A", "B", "C")):
                          wqs[nm] = sp.tile([128, n_dt, 128], FP8,
                                            name=f"{R}wq{nm}_{p}", tag=f"wq{nm}")
                          wks[nm] = sp.tile([128, n_dt, 128], FP8,
                                            name=f"{R}wk{nm}_{p}", tag=f"wk{nm}")
                          nc.gpsimd.dma_start(
                              out=wqs[nm],
                              in_=wq8[nm][p].rearrange("r (t m) -> r t m", m=128))
                          nc.sync.dma_start(
                              out=wks[nm],
                              in_=wk8[nm][p].rearrange("r (t m) -> r t m", m=128))
                  else:
                      wq_sb = sp.tile([128, n_dt, 128], F32R, name=f"{R}wq_{p}",
                                      tag="wqA")
                      wk_sb = sp.tile([128, n_dt, 128], F32R, name=f"{R}wk_{p}",
                                      tag="wkA")
                      wq_r = wqT.rearrange("(t r) m -> r t m", r=128)
                      wk_r = wkT.rearrange("(t r) m -> r t m", r=128)
                      nc.gpsimd.dma_start(out=wq_sb,
                                          in_=wq_r[:, :, 128 * p:128 * (p + 1)])
                      nc.sync.dma_start(out=wk_sb,
                                        in_=wk_r[:, :, 128 * p:128 * (p + 1)])

                  # Q/K projections (f32r) -> fp8 evacuation (x16)
                  for c4 in range(n_ch):
                      g = ps.tile([128, 2, 512], F32, name=f"{R}pqk_{p}_{c4}",
                                  tag=("sg", "pp")[c4 % 2],
                                  bufs=(2 if c4 % 2 == 0 else 1))
                      psq, psk = g[:, 0, :], g[:, 1, :]
                      cs = slice(512 * c4, 512 * (c4 + 1))
                      if QKV_MODE == "dr3":
                          prods = (("A", xh), ("B", xl), ("C", xh))
                          for ps_, ws_ in ((psq, wqs), (psk, wks)):
                              i = 0
                              for nm, xsrc in prods:
                                  for t2 in range(n_dt // 2):
                                      nc.tensor.matmul(
                                          ps_, ws_[nm][:, 2 * t2:2 * t2 + 2, :],
                                          xsrc[:, 2 * t2:2 * t2 + 2, cs],
                                          start=(i == 0),
                                          stop=(i == 3 * (n_dt // 2) - 1),
                                          perf_mode=DR)
                                      i += 1
                          evsc = 2.0 ** -9
                      else:
                          for ps_, ws_ in ((psq, wq_sb), (psk, wk_sb)):
                              for t in range(n_dt):
                                  nc.tensor.matmul(ps_, ws_[:, t, :], xt[:, t, cs],
                                                   start=(t == 0),
                                                   stop=(t == n_dt - 1))
                          evsc = 16.0
                      nc.vector.tensor_scalar_mul(
                          qt_all[:, pr, 512 * c4:512 * (c4 + 1)], psq, evsc)
                      # K quadrant evacuations: head A rows 0:64, head B 64:128
                      ja, jb = 4 * c4, 4 * (c4 + 1)
                      nc.vector.tensor_scalar_mul(
                          kt_dr[0:64, ja:jb, 0, 0, :],
                          psk[0:64, :].rearrange("p (j m) -> p j m", m=128), evsc)
                      nc.vector.tensor_scalar_mul(
                          kt_dr[64:128, ja:jb, 1, 0, :],
                          psk[64:128, :].rearrange("p (j m) -> p j m", m=128), evsc)

                  # ---- attention over chunks ----
                  qplane = qt_all[:, pr, :]
                  for c4 in range(n_ch):
                      q0 = 512 * c4
                      ctxA = ps.tile([DH + 1, 512], F32, name=f"{R}cA_{p}_{c4}", tag="ctxA")
                      ctxB = ps.tile([DH + 1, 512], F32, name=f"{R}cB_{p}_{c4}", tag="ctxB")
                      nj = 4 * c4 + 4
                      pending = None
                      for j in range(nj):
                          m = j - 4 * c4
                          n0 = 128 * m if m >= 0 else 0
                          w = 512 - n0
                          sg = ps.tile([128, 2, 512], F32, name=f"{R}sg_{p}_{c4}_{j}",
                                       tag="sg", bufs=2)
                          o1 = TRI0 if m >= 0 else ZERO0
                          rhs = dr_rhs(qplane, q0 + n0, o1, w)
                          for h in (0, 1):
                              nc.tensor.matmul(sg[:, h, n0:512],
                                               kt_dr[:, j, h, :, :], rhs,
                                               start=True, stop=True, perf_mode=DR)
                          pt = sp.tile([128, 2, 512], BF16, name=f"{R}pt_{p}_{c4}_{j}",
                                       tag="pt", bufs=4)
                          nc.scalar.activation(out=pt[:, :, n0:512],
                                               in_=sg[:, :, n0:512],
                                               func=AFT.Exp, scale=0.125 / 256.0)
                          if pending is not None:
                              _emit_pv(nc, v_sb, ctxA, ctxB, p, pending, nj)
                          pending = (j, pt, n0)
                      _emit_pv(nc, v_sb, ctxA, ctxB, p, pending, nj)

                      # normalize (baseline flow: the custom DVE ops only
                      # handle full-tile APs on HW): evacuate ctx + denom to
                      # SBUF, reciprocal, partition broadcast, in-place mult
                      for head, cpsum in ((0, ctxA), (1, ctxB)):
                          r_i = 2 * c4 + head
                          hs = slice(64 * head, 64 * head + 64)
                          nc.vector.tensor_copy(
                              out=ctx_all[hs, p, q0:q0 + 512],
                              in_=cpsum[0:DH, :])
                          dn1 = sp.tile([1, 512], F32, name=f"{R}dn_{p}_{r_i}",
                                        tag="dn", bufs=2)
                          nc.vector.tensor_copy(out=dn1, in_=cpsum[DH:DH + 1, :])
                          rb1 = sp.tile([1, 512], F32, name=f"{R}rc_{p}_{r_i}",
                                        tag="rc", bufs=2)
                          nc.vector.reciprocal_approx_fast(out=rb1, in_=dn1)
                          rb = sp.tile([128, 512], F32, name=f"{R}rb_{p}_{r_i}",
                                       tag="rb", bufs=2)
                          nc.gpsimd.partition_broadcast(rb, rb1)
                          cslice = ctx_all[hs, p, q0:q0 + 512]
                          nc.vector.tensor_mul(cslice, cslice, rb[hs, :])

              # ---- O projection (bf16, partial: contracts local 512 dims) ----
              wo_sb = sp.tile([128, pairs, d], BF16, name=f"{R}wo_sb", tag="wvo")
              for ct in range(pairs):
                  nc.sync.dma_start(out=wo_sb[:, ct, :],
                                    in_=woT[128 * ct:128 * (ct + 1), :])
              for st_i in range(n_kt):
                  ot = sp.tile([128, d], F16, name=f"{R}ot_{st_i}",
                               tag="ot", bufs=3)
                  if st_i % 2 == 0:
                      og = ps.tile([128, 2, 512], F32, name=f"{R}po_{st_i}",
                                   tag=("pp", "sg")[(st_i // 2) % 2],
                                   bufs=(1 if (st_i // 2) % 2 == 0 else 2))
                  for oc in range(n_oc):
                      pso = og[:, (2 * st_i + oc) % 2, :]
                      for ct in range(pairs):
                          nc.tensor.matmul(pso,
                                           ctx_all[:, ct, 128 * st_i:128 * (st_i + 1)],
                                           wo_sb[:, ct, 512 * oc:512 * (oc + 1)],
                                           start=(ct == 0), stop=(ct == pairs - 1))
                      nc.vector.tensor_copy(out=ot[:, 512 * oc:512 * (oc + 1)],
                                            in_=pso)
                  oeng = nc.sync if st_i % 2 == 0 else nc.gpsimd
                  oeng.dma_start(out=out[128 * st_i:128 * (st_i + 1), :], in_=ot)

    nc.compile()
    return nc


def _emit_pv(nc, v_sb, ctxA, ctxB, p, pending, nj):
    j, pt, n0 = pending
    start = (j == 0)
    stop = (j == nj - 1)
    nc.tensor.matmul(ctxA[:, n0:512], v_sb[:, j, 2 * p, :], pt[:, 0, n0:512],
                     start=start, stop=stop)
    nc.tensor.matmul(ctxB[:, n0:512], v_sb[:, j, 2 * p + 1, :], pt[:, 1, n0:512],
                     start=start, stop=stop)


def make_dg():
    m = np.zeros((128, 128), np.float32)
    np.fill_diagonal(m, -MV)
    return m.astype(E4NP)


def make_t240():
    k = np.arange(128)[:, None]
    q = np.arange(128)[None, :]
    return (MV * (q < k)).astype(np.float32).astype(E4NP)


def shard_inputs(in_features, q_weight, k_weight, v_weight, o_weight):
    """-> list of 8 per-core input dicts."""
    dgm = make_dg()
    t240 = make_t240()
    maps = []
    for c in range(N_CORES):
        b, g = divmod(c, 2)
        hs = slice(DL * g, DL * (g + 1))   # local head dims in the full D
        xt = np.ascontiguousarray(in_features[b].T)
        m = {
            "woT": np.ascontiguousarray(o_weight[:, hs].T.astype(BFNP)),
            "dg": dgm,
            "t240": t240,
        }
        if QKV_MODE == "dr3":
            xh = (xt * 16.0).astype(E4NP)
            xl = ((xt * 16.0 - xh.astype(np.float32)) * 16.0).astype(E4NP)
            m["xh8"] = xh
            m["xl8"] = xl
        else:
            m["xT"] = xt
            m["wqT"] = np.ascontiguousarray(q_weight[hs, :].T)
            m["wkT"] = np.ascontiguousarray(k_weight[hs, :].T)
            m["wvT"] = np.ascontiguousarray(v_weight[hs, :].T)
        for wname, w in ((("wq8", q_weight), ("wk8", k_weight),
                          ("wv8", v_weight)) if QKV_MODE == "dr3" else ()):
            wt = np.ascontiguousarray(w[hs, :].T).astype(np.float32)  # [D, 512]
            A = (wt * 512.0).astype(E4NP)
            Bm = (wt * 32.0).astype(E4NP)
            C = (wt * 512.0 - A.astype(np.float32)).astype(E4NP)
            if wname in ("wq8", "wk8"):
                # -> [pairs, 128 rows, n_dt*128]: SBUF layout, contiguous per pair
                def tile_w(a):
                    # a: [D, 512] -> pair p slice cols 128p:128p+128,
                    # rows (t,r) -> [p, r, t*128+m]
                    a4 = a.reshape(D // 128, 128, 4, 128)      # [t, r, p, m]
                    return np.ascontiguousarray(
                        a4.transpose(2, 1, 0, 3).reshape(4, 128, -1))
                m[wname + "A"] = tile_w(A)
                m[wname + "B"] = tile_w(Bm)
                m[wname + "C"] = tile_w(C)
            else:
                m[wname + "A"] = A
                m[wname + "B"] = Bm
                m[wname + "C"] = C
        maps.append(m)
    return maps


def gather_output(results):
    """results: list of 8 dicts with 'out' [S, D] partials -> [B, S, D]."""
    return np.stack([results[2 * b]["out"].astype(np.float32)
                     + results[2 * b + 1]["out"].astype(np.float32)
                     for b in range(B)])


_nc_cache = {}


def kernel(in_features, q_weight, k_weight, v_weight, o_weight):
    from concourse.bass_utils import run_bass_kernel_spmd
    if "nc" not in _nc_cache:
        _nc_cache["nc"] = build_nc()
    nc = _nc_cache["nc"]
    in_maps = shard_inputs(np.asarray(in_features, dtype=np.float32),
                           np.asarray(q_weight, dtype=np.float32),
                           np.asarray(k_weight, dtype=np.float32),
                           np.asarray(v_weight, dtype=np.float32),
                           np.asarray(o_weight, dtype=np.float32))
    res = run_bass_kernel_spmd(nc, in_maps, core_ids=list(range(N_CORES)))
    return gather_output(res.results)
